# revision 8
# baseline (speedup 1.0000x reference)
"""Trainium2 kernel for the AttackHead GNN edge-scoring module.

Strategy (data-parallel, one batch element per NeuronCore, 8 cores):
  - Host converts node embeddings to bf16 and pre-shuffles them into the
    SBUF gather layout (two regions so int16 gather indices stay < 32768).
  - Host partitions each batch's edges into 4 groups by which table region
    the src/tgt index falls in, padding each group to a fleet-wide static
    size so all 8 cores run one SPMD program.
  - Device: resident bf16 table in SBUF, `dma_gather` (transpose mode)
    pulls src/tgt embeddings feature-major, TensorE runs both 2-layer MLPs
    edge-major, outputs [5, n] = (edge_logit, 4 army logits) per edge.
  - Host un-permutes outputs and applies the self-edge -100 mask.
"""

import numpy as np
import ml_dtypes

import concourse.bass as bass
import concourse.bacc as bacc
import concourse.mybir as mybir
import concourse.tile as tile
from concourse.vector_clock import ScopedClock
from concourse.bass_utils import run_bass_kernel_spmd

# ---------------------------------------------------------------- constants
B, N, D, E = 8, 50000, 128, 100000
P = 128
NA = 32768                    # region-A rows (int16-addressable)
RANKS_A = NA // P             # 256
RANKS_B = -(-(N - NA) // P)   # 135
NB = RANKS_B * P              # 17280 (padded)
SUB = 512                     # matmul free-dim tile
GC = 6144                     # gather chunk (edges per dma_gather)
OC = 2048                     # output block per DMA
N_CORES = 8
USE_SBUF_TABLE = True

BF16 = ml_dtypes.bfloat16


# ------------------------------------------------------------- tile patch
def _patched_drain_and_barrier(self, tick_clock, wait_clock):
    """Walrus in this toolchain only accepts one sync-wait per CTRL
    instruction; split the Tile tail-drain's waits across extra drains."""
    drain_inst = self.nc.sync.drain()
    wait_clock.add_sem_waits(
        drain_inst.ins, ScopedClock({None: tick_clock.global_clock})
    )
    si = drain_inst.ins.sync_info
    waits = list(si.on_wait or []) if si is not None else []
    if len(waits) > 1:
        si.on_wait = waits[:1]
        for w in waits[1:]:
            extra = self.nc.sync.drain()
            esi = extra.ins.sync_info
            if esi is None:
                extra.ins.sync_info = mybir.SyncInfo(on_wait=[w], on_update=[])
            else:
                esi.on_wait = [w]
    self.nc.all_engine_barrier()
    assert self.sems is not None
    popped = self.nc._tile_sem_poison_stack.pop()
    assert popped is self._sem_poison
    self.nc.clear_and_free_semaphores(list(self.sems.allocated().values()))
    self.nc.all_engine_barrier()


tile.TileContext._drain_and_barrier = _patched_drain_and_barrier


# ------------------------------------------------------------ device build
def build_kernel(group_sizes, ranks_a=RANKS_A, ranks_b=RANKS_B, d=D,
                 sub=SUB, gc=GC, oc=OC, use_sbuf_table=USE_SBUF_TABLE):
    """group_sizes: static per-group padded edge counts (multiples of sub).
    Group regions: 0=(A,A) 1=(A,B) 2=(B,A) 3=(B,B)."""
    dt = mybir.dt
    tp = sum(group_sizes)
    na, nb = ranks_a * P, ranks_b * P

    nc = bacc.Bacc(dynamic_dma_scratch_size=49152)
    if use_sbuf_table:
        tabA_d = nc.declare_dram_parameter("tabA", [P, ranks_a * d], dt.bfloat16, isOutput=False)
        tabB_d = nc.declare_dram_parameter("tabB", [P, ranks_b * d], dt.bfloat16, isOutput=False)
    else:
        tabA_d = nc.declare_dram_parameter("tabA", [na, d], dt.bfloat16, isOutput=False)
        tabB_d = nc.declare_dram_parameter("tabB", [nb, d], dt.bfloat16, isOutput=False)
    isrc_d = nc.declare_dram_parameter("isrc", [P, tp // 16], dt.int16, isOutput=False)
    itgt_d = nc.declare_dram_parameter("itgt", [P, tp // 16], dt.int16, isOutput=False)
    wse_d = nc.declare_dram_parameter("wse", [d, d], dt.bfloat16, isOutput=False)
    wte_d = nc.declare_dram_parameter("wte", [d, d], dt.bfloat16, isOutput=False)
    wsa_d = nc.declare_dram_parameter("wsa", [d, d], dt.bfloat16, isOutput=False)
    wta_d = nc.declare_dram_parameter("wta", [d, d], dt.bfloat16, isOutput=False)
    w2e_d = nc.declare_dram_parameter("w2e", [d, 5], dt.bfloat16, isOutput=False)
    w2a_d = nc.declare_dram_parameter("w2a", [d, 5], dt.bfloat16, isOutput=False)
    b1e_d = nc.declare_dram_parameter("b1e", [d, 1], dt.float32, isOutput=False)
    b1a_d = nc.declare_dram_parameter("b1a", [d, 1], dt.float32, isOutput=False)
    b2_d = nc.declare_dram_parameter("b2", [5, 1], dt.float32, isOutput=False)
    out_d = nc.declare_dram_parameter("out", [5, tp], dt.float32, isOutput=True)

    relu = mybir.ActivationFunctionType.Relu
    ident = mybir.ActivationFunctionType.Identity

    with tile.TileContext(nc) as tc:
        with tc.tile_pool(name="tabp", bufs=1) as tabp, \
             tc.tile_pool(name="wp", bufs=1) as wp, \
             tc.tile_pool(name="idxp", bufs=2) as idxp, \
             tc.tile_pool(name="gp", bufs=2) as gp, \
             tc.tile_pool(name="hp", bufs=3) as hp, \
             tc.tile_pool(name="op", bufs=2) as op, \
             tc.tile_pool(name="pp1", bufs=4, space="PSUM") as pp1, \
             tc.tile_pool(name="pp2", bufs=2, space="PSUM") as pp2:

            if use_sbuf_table:
                tabA_t = tabp.tile([P, ranks_a * d], dt.bfloat16, tag="tabA")
                tabB_t = tabp.tile([P, ranks_b * d], dt.bfloat16, tag="tabB")
                nc.sync.dma_start(out=tabA_t[:], in_=tabA_d[:])
                nc.sync.dma_start(out=tabB_t[:], in_=tabB_d[:])

            wse_t = wp.tile([d, d], dt.bfloat16, tag="wse")
            wte_t = wp.tile([d, d], dt.bfloat16, tag="wte")
            wsa_t = wp.tile([d, d], dt.bfloat16, tag="wsa")
            wta_t = wp.tile([d, d], dt.bfloat16, tag="wta")
            w2e_t = wp.tile([d, 5], dt.bfloat16, tag="w2e")
            w2a_t = wp.tile([d, 5], dt.bfloat16, tag="w2a")
            b1e_t = wp.tile([d, 1], dt.float32, tag="b1e")
            b1a_t = wp.tile([d, 1], dt.float32, tag="b1a")
            b2_t = wp.tile([5, 1], dt.float32, tag="b2")
            for t, src in [(wse_t, wse_d), (wte_t, wte_d), (wsa_t, wsa_d),
                           (wta_t, wta_d), (w2e_t, w2e_d), (w2a_t, w2a_d),
                           (b1e_t, b1e_d), (b1a_t, b1a_d), (b2_t, b2_d)]:
                nc.sync.dma_start(out=t[:], in_=src[:])

            regions = [(0, 0), (0, 1), (1, 0), (1, 1)]
            off = 0
            for g, S in enumerate(group_sizes):
                if S == 0:
                    continue
                if use_sbuf_table:
                    src_tab = tabA_t if regions[g][0] == 0 else tabB_t
                    tgt_tab = tabA_t if regions[g][1] == 0 else tabB_t
                else:
                    src_tab = tabA_d if regions[g][0] == 0 else tabB_d
                    tgt_tab = tabA_d if regions[g][1] == 0 else tabB_d
                for c in range(0, S, gc):
                    n = min(gc, S - c)
                    c0 = off + c
                    ist = idxp.tile([P, n // 16], dt.int16, tag="isrc")
                    itt = idxp.tile([P, n // 16], dt.int16, tag="itgt")
                    nc.sync.dma_start(out=ist[:], in_=isrc_d[:, c0 // 16:(c0 + n) // 16])
                    nc.sync.dma_start(out=itt[:], in_=itgt_d[:, c0 // 16:(c0 + n) // 16])
                    gs = gp.tile([P, 1, n], dt.bfloat16, tag="gs")
                    gt = gp.tile([P, 1, n], dt.bfloat16, tag="gt")
                    kw = dict(sbuf_tokens_per_rank=P, sbuf_free_dim_per_rank=2 * d) \
                        if use_sbuf_table else {}
                    nc.gpsimd.dma_gather(gs[:], src_tab[:], ist[:], num_idxs=n,
                                         num_idxs_reg=n, elem_size=d,
                                         transpose=True, single_packet=False, **kw)
                    nc.gpsimd.dma_gather(gt[:], tgt_tab[:], itt[:], num_idxs=n,
                                         num_idxs_reg=n, elem_size=d,
                                         transpose=True, single_packet=False, **kw)
                    for j0 in range(0, n, oc):
                        blk = min(oc, n - j0)
                        oct_ = op.tile([5, oc], dt.float32, tag="oc")
                        for j in range(j0, j0 + blk, sub):
                            m = min(sub, j0 + blk - j)
                            ps = pp1.tile([P, sub], dt.float32, tag="ps")
                            nc.tensor.matmul(out=ps[:, :m], lhsT=wse_t[:],
                                             rhs=gs[:, 0, j:j + m], start=True, stop=False)
                            nc.tensor.matmul(out=ps[:, :m], lhsT=wte_t[:],
                                             rhs=gt[:, 0, j:j + m], start=False, stop=True)
                            he = hp.tile([P, sub], dt.bfloat16, tag="he")
                            nc.scalar.activation(out=he[:, :m], in_=ps[:, :m],
                                                 func=relu, bias=b1e_t[:])
                            ps2 = pp1.tile([P, sub], dt.float32, tag="ps")
                            nc.tensor.matmul(out=ps2[:, :m], lhsT=wsa_t[:],
                                             rhs=gs[:, 0, j:j + m], start=True, stop=False)
                            nc.tensor.matmul(out=ps2[:, :m], lhsT=wta_t[:],
                                             rhs=gt[:, 0, j:j + m], start=False, stop=True)
                            ha = hp.tile([P, sub], dt.bfloat16, tag="ha")
                            nc.scalar.activation(out=ha[:, :m], in_=ps2[:, :m],
                                                 func=relu, bias=b1a_t[:])
                            ps3 = pp2.tile([5, sub], dt.float32, tag="ps3")
                            nc.tensor.matmul(out=ps3[:, :m], lhsT=w2e_t[:],
                                             rhs=he[:, :m], start=True, stop=False)
                            nc.tensor.matmul(out=ps3[:, :m], lhsT=w2a_t[:],
                                             rhs=ha[:, :m], start=False, stop=True)
                            nc.scalar.activation(out=oct_[:, j - j0:j - j0 + m],
                                                 in_=ps3[:, :m], func=ident, bias=b2_t[:])
                        nc.sync.dma_start(out=out_d[:, c0 + j0:c0 + j0 + blk],
                                          in_=oct_[:, :blk])
                off += S
    nc.compile()
    return nc


# --------------------------------------------------------------- host prep
def _wrap_idx(stream):
    """int16 stream -> [128, len/16] SBUF wrap layout (16-wrap, 8x replicated)."""
    a = stream.reshape(-1, 16).T  # [16, n/16]
    return np.ascontiguousarray(np.tile(a, (8, 1)))


def _shuffle_table(tab, ranks):
    """[ranks*128, d] -> [128, ranks*d] with [p, r*d:(r+1)*d] = row r*128+p."""
    d = tab.shape[1]
    return np.ascontiguousarray(
        tab.reshape(ranks, P, d).transpose(1, 0, 2).reshape(P, ranks * d)
    )


def kernel(node_embeddings, action_edges, ew1, eb1, ew2, eb2, aw1, ab1, aw2, ab2):
    node_embeddings = np.asarray(node_embeddings)
    action_edges = np.asarray(action_edges)
    b, n, d = node_embeddings.shape
    e = action_edges.shape[1]

    src = np.clip(action_edges[..., 0], 0, n - 1).astype(np.int64)
    tgt = np.clip(action_edges[..., 1], 0, n - 1).astype(np.int64)

    # 4 groups by (src-half, tgt-half); stable order within group
    gid = (src >= NA).astype(np.int8) * 2 + (tgt >= NA).astype(np.int8)
    orders = [np.argsort(gid[i], kind="stable") for i in range(b)]
    counts = np.stack([np.bincount(gid[i], minlength=4) for i in range(b)])
    group_sizes = [int(-(-counts[:, g].max() // SUB) * SUB) for g in range(4)]
    tp = sum(group_sizes)

    # per-core padded index streams (adjusted into region-local row ids)
    in_maps = []
    w = {
        "wse": np.ascontiguousarray(ew1[:d]).astype(BF16),
        "wte": np.ascontiguousarray(ew1[d:]).astype(BF16),
        "wsa": np.ascontiguousarray(aw1[:d]).astype(BF16),
        "wta": np.ascontiguousarray(aw1[d:]).astype(BF16),
        "w2e": np.concatenate([np.asarray(ew2), np.zeros((d, 4), np.float32)], axis=1).astype(BF16),
        "w2a": np.concatenate([np.zeros((d, 1), np.float32), np.asarray(aw2)], axis=1).astype(BF16),
        "b1e": np.asarray(eb1, np.float32).reshape(d, 1),
        "b1a": np.asarray(ab1, np.float32).reshape(d, 1),
        "b2": np.concatenate([np.asarray(eb2), np.asarray(ab2)]).astype(np.float32).reshape(5, 1),
    }
    for i in range(b):
        emb = node_embeddings[i].astype(BF16)
        tab_a = emb[:NA]
        tab_b = np.zeros((NB, d), BF16)
        tab_b[: n - NA] = emb[NA:]
        if USE_SBUF_TABLE:
            tab_a = _shuffle_table(tab_a, RANKS_A)
            tab_b = _shuffle_table(tab_b, RANKS_B)
        s_perm, t_perm = src[i][orders[i]], tgt[i][orders[i]]
        s_stream = np.zeros(tp, np.int16)
        t_stream = np.zeros(tp, np.int16)
        off = 0
        pos = 0
        for g in range(4):
            cnt = int(counts[i, g])
            s_g = s_perm[pos:pos + cnt]
            t_g = t_perm[pos:pos + cnt]
            s_stream[off:off + cnt] = (s_g - (NA if g >= 2 else 0)).astype(np.int16)
            t_stream[off:off + cnt] = (t_g - (NA if g % 2 else 0)).astype(np.int16)
            off += group_sizes[g]
            pos += cnt
        in_maps.append({
            "tabA": tab_a, "tabB": tab_b,
            "isrc": _wrap_idx(s_stream), "itgt": _wrap_idx(t_stream),
            **w,
        })

    nc = build_kernel(group_sizes)
    res = run_bass_kernel_spmd(nc, in_maps, core_ids=list(range(N_CORES)))

    edge_logits = np.empty((b, e), np.float32)
    army_logits = np.empty((b, e, 4), np.float32)
    for i in range(b):
        data = res.results[i]["out"]  # [5, tp]
        off = 0
        pos = 0
        for g in range(4):
            cnt = int(counts[i, g])
            ids = orders[i][pos:pos + cnt]
            edge_logits[i, ids] = data[0, off:off + cnt]
            army_logits[i, ids, :] = data[1:5, off:off + cnt].T
            off += group_sizes[g]
            pos += cnt

    s_raw = action_edges[..., 0]
    t_raw = action_edges[..., 1]
    invalid_self = (s_raw == t_raw) & (s_raw >= 0) & (t_raw >= 0)
    edge_logits -= invalid_self.astype(np.float32) * 100.0
    return edge_logits, army_logits


# revision 9
# speedup vs baseline: 1.1978x; 1.1978x over previous
"""Trainium2 kernel for the AttackHead GNN edge-scoring module.

Strategy (data-parallel, one batch element per NeuronCore, 8 cores):
  - Host converts node embeddings to bf16 and pre-shuffles them into the
    SBUF gather layout (two regions so int16 gather indices stay < 32768).
  - Host partitions each batch's edges into 4 groups by which table region
    the src/tgt index falls in, padding each group to a fleet-wide static
    size so all 8 cores run one SPMD program.
  - Device: resident bf16 table in SBUF, `dma_gather` (transpose mode)
    pulls src/tgt embeddings feature-major, TensorE runs both 2-layer MLPs
    edge-major, outputs [5, n] = (edge_logit, 4 army logits) per edge.
  - Host un-permutes outputs and applies the self-edge -100 mask.
"""

import numpy as np
import ml_dtypes

import concourse.bass as bass
import concourse.bacc as bacc
import concourse.mybir as mybir
import concourse.tile as tile
from concourse.vector_clock import ScopedClock
from concourse.bass_utils import run_bass_kernel_spmd

# ---------------------------------------------------------------- constants
B, N, D, E = 8, 50000, 128, 100000
P = 128
NA = 32768                    # region-A rows (int16-addressable)
RANKS_A = NA // P             # 256
RANKS_B = -(-(N - NA) // P)   # 135
NB = RANKS_B * P              # 17280 (padded)
SUB = 512                     # matmul free-dim tile
GC = 6144                     # gather chunk (edges per dma_gather)
OC = 2048                     # output block per DMA
N_CORES = 8
USE_SBUF_TABLE = True

BF16 = ml_dtypes.bfloat16


# ------------------------------------------------------------- tile patch
def _patched_drain_and_barrier(self, tick_clock, wait_clock):
    """Walrus in this toolchain only accepts one sync-wait per CTRL
    instruction; split the Tile tail-drain's waits across extra drains."""
    drain_inst = self.nc.sync.drain()
    wait_clock.add_sem_waits(
        drain_inst.ins, ScopedClock({None: tick_clock.global_clock})
    )
    si = drain_inst.ins.sync_info
    waits = list(si.on_wait or []) if si is not None else []
    if len(waits) > 1:
        si.on_wait = waits[:1]
        for w in waits[1:]:
            extra = self.nc.sync.drain()
            esi = extra.ins.sync_info
            if esi is None:
                extra.ins.sync_info = mybir.SyncInfo(on_wait=[w], on_update=[])
            else:
                esi.on_wait = [w]
    self.nc.all_engine_barrier()
    assert self.sems is not None
    popped = self.nc._tile_sem_poison_stack.pop()
    assert popped is self._sem_poison
    self.nc.clear_and_free_semaphores(list(self.sems.allocated().values()))
    self.nc.all_engine_barrier()


tile.TileContext._drain_and_barrier = _patched_drain_and_barrier


# ------------------------------------------------------------ device build
def build_kernel(group_sizes, ranks_a=RANKS_A, ranks_b=RANKS_B, d=D,
                 sub=SUB, gc=GC, oc=OC, use_sbuf_table=USE_SBUF_TABLE):
    """group_sizes: static per-group padded edge counts (multiples of sub).
    Group regions: 0=(A,A) 1=(A,B) 2=(B,A) 3=(B,B)."""
    dt = mybir.dt
    tp = sum(group_sizes)
    na, nb = ranks_a * P, ranks_b * P

    nc = bacc.Bacc()
    if use_sbuf_table:
        tabA_d = nc.declare_dram_parameter("tabA", [P, ranks_a * d], dt.bfloat16, isOutput=False)
        tabB_d = nc.declare_dram_parameter("tabB", [P, ranks_b * d], dt.bfloat16, isOutput=False)
    else:
        tabA_d = nc.declare_dram_parameter("tabA", [na, d], dt.bfloat16, isOutput=False)
        tabB_d = nc.declare_dram_parameter("tabB", [nb, d], dt.bfloat16, isOutput=False)
    isrc_d = nc.declare_dram_parameter("isrc", [P, tp // 16], dt.int16, isOutput=False)
    itgt_d = nc.declare_dram_parameter("itgt", [P, tp // 16], dt.int16, isOutput=False)
    wse_d = nc.declare_dram_parameter("wse", [d, d], dt.bfloat16, isOutput=False)
    wte_d = nc.declare_dram_parameter("wte", [d, d], dt.bfloat16, isOutput=False)
    wsa_d = nc.declare_dram_parameter("wsa", [d, d], dt.bfloat16, isOutput=False)
    wta_d = nc.declare_dram_parameter("wta", [d, d], dt.bfloat16, isOutput=False)
    w2e_d = nc.declare_dram_parameter("w2e", [d, 5], dt.bfloat16, isOutput=False)
    w2a_d = nc.declare_dram_parameter("w2a", [d, 5], dt.bfloat16, isOutput=False)
    b1e_d = nc.declare_dram_parameter("b1e", [d, 1], dt.float32, isOutput=False)
    b1a_d = nc.declare_dram_parameter("b1a", [d, 1], dt.float32, isOutput=False)
    b2_d = nc.declare_dram_parameter("b2", [5, 1], dt.float32, isOutput=False)
    out_d = nc.declare_dram_parameter("out", [5, tp], dt.float32, isOutput=True)

    relu = mybir.ActivationFunctionType.Relu
    ident = mybir.ActivationFunctionType.Identity

    with tile.TileContext(nc) as tc:
        with tc.tile_pool(name="tabp", bufs=1) as tabp, \
             tc.tile_pool(name="wp", bufs=1) as wp, \
             tc.tile_pool(name="idxp", bufs=2) as idxp, \
             tc.tile_pool(name="gp", bufs=2) as gp, \
             tc.tile_pool(name="hp", bufs=3) as hp, \
             tc.tile_pool(name="op", bufs=2) as op, \
             tc.tile_pool(name="pp1", bufs=4, space="PSUM") as pp1, \
             tc.tile_pool(name="pp2", bufs=2, space="PSUM") as pp2:

            if use_sbuf_table:
                tabA_t = tabp.tile([P, ranks_a * d], dt.bfloat16, tag="tabA")
                tabB_t = tabp.tile([P, ranks_b * d], dt.bfloat16, tag="tabB")
                nc.sync.dma_start(out=tabA_t[:], in_=tabA_d[:])
                nc.sync.dma_start(out=tabB_t[:], in_=tabB_d[:])

            wse_t = wp.tile([d, d], dt.bfloat16, tag="wse")
            wte_t = wp.tile([d, d], dt.bfloat16, tag="wte")
            wsa_t = wp.tile([d, d], dt.bfloat16, tag="wsa")
            wta_t = wp.tile([d, d], dt.bfloat16, tag="wta")
            w2e_t = wp.tile([d, 5], dt.bfloat16, tag="w2e")
            w2a_t = wp.tile([d, 5], dt.bfloat16, tag="w2a")
            b1e_t = wp.tile([d, 1], dt.float32, tag="b1e")
            b1a_t = wp.tile([d, 1], dt.float32, tag="b1a")
            b2_t = wp.tile([5, 1], dt.float32, tag="b2")
            for t, src in [(wse_t, wse_d), (wte_t, wte_d), (wsa_t, wsa_d),
                           (wta_t, wta_d), (w2e_t, w2e_d), (w2a_t, w2a_d),
                           (b1e_t, b1e_d), (b1a_t, b1a_d), (b2_t, b2_d)]:
                nc.sync.dma_start(out=t[:], in_=src[:])

            regions = [(0, 0), (0, 1), (1, 0), (1, 1)]
            off = 0
            for g, S in enumerate(group_sizes):
                if S == 0:
                    continue
                if use_sbuf_table:
                    src_tab = tabA_t if regions[g][0] == 0 else tabB_t
                    tgt_tab = tabA_t if regions[g][1] == 0 else tabB_t
                else:
                    src_tab = tabA_d if regions[g][0] == 0 else tabB_d
                    tgt_tab = tabA_d if regions[g][1] == 0 else tabB_d
                for c in range(0, S, gc):
                    n = min(gc, S - c)
                    c0 = off + c
                    ist = idxp.tile([P, n // 16], dt.int16, tag="isrc")
                    itt = idxp.tile([P, n // 16], dt.int16, tag="itgt")
                    nc.sync.dma_start(out=ist[:], in_=isrc_d[:, c0 // 16:(c0 + n) // 16])
                    nc.sync.dma_start(out=itt[:], in_=itgt_d[:, c0 // 16:(c0 + n) // 16])
                    gs = gp.tile([P, 1, n], dt.bfloat16, tag="gs")
                    gt = gp.tile([P, 1, n], dt.bfloat16, tag="gt")
                    kw = dict(sbuf_tokens_per_rank=P, sbuf_free_dim_per_rank=2 * d) \
                        if use_sbuf_table else {}
                    nc.gpsimd.dma_gather(gs[:], src_tab[:], ist[:], num_idxs=n,
                                         num_idxs_reg=n, elem_size=d,
                                         transpose=True, single_packet=False, **kw)
                    nc.gpsimd.dma_gather(gt[:], tgt_tab[:], itt[:], num_idxs=n,
                                         num_idxs_reg=n, elem_size=d,
                                         transpose=True, single_packet=False, **kw)
                    for j0 in range(0, n, oc):
                        blk = min(oc, n - j0)
                        oct_ = op.tile([5, oc], dt.float32, tag="oc")
                        for j in range(j0, j0 + blk, sub):
                            m = min(sub, j0 + blk - j)
                            ps = pp1.tile([P, sub], dt.float32, tag="ps")
                            nc.tensor.matmul(out=ps[:, :m], lhsT=wse_t[:],
                                             rhs=gs[:, 0, j:j + m], start=True, stop=False)
                            nc.tensor.matmul(out=ps[:, :m], lhsT=wte_t[:],
                                             rhs=gt[:, 0, j:j + m], start=False, stop=True)
                            he = hp.tile([P, sub], dt.bfloat16, tag="he")
                            nc.scalar.activation(out=he[:, :m], in_=ps[:, :m],
                                                 func=relu, bias=b1e_t[:])
                            ps2 = pp1.tile([P, sub], dt.float32, tag="ps")
                            nc.tensor.matmul(out=ps2[:, :m], lhsT=wsa_t[:],
                                             rhs=gs[:, 0, j:j + m], start=True, stop=False)
                            nc.tensor.matmul(out=ps2[:, :m], lhsT=wta_t[:],
                                             rhs=gt[:, 0, j:j + m], start=False, stop=True)
                            ha = hp.tile([P, sub], dt.bfloat16, tag="ha")
                            nc.scalar.activation(out=ha[:, :m], in_=ps2[:, :m],
                                                 func=relu, bias=b1a_t[:])
                            ps3 = pp2.tile([5, sub], dt.float32, tag="ps3")
                            nc.tensor.matmul(out=ps3[:, :m], lhsT=w2e_t[:],
                                             rhs=he[:, :m], start=True, stop=False)
                            nc.tensor.matmul(out=ps3[:, :m], lhsT=w2a_t[:],
                                             rhs=ha[:, :m], start=False, stop=True)
                            nc.scalar.activation(out=oct_[:, j - j0:j - j0 + m],
                                                 in_=ps3[:, :m], func=ident, bias=b2_t[:])
                        nc.sync.dma_start(out=out_d[:, c0 + j0:c0 + j0 + blk],
                                          in_=oct_[:, :blk])
                off += S
    nc.compile()
    return nc


# --------------------------------------------------------------- host prep
def _wrap_idx(stream):
    """int16 stream -> [128, len/16] SBUF wrap layout (16-wrap, 8x replicated)."""
    a = stream.reshape(-1, 16).T  # [16, n/16]
    return np.ascontiguousarray(np.tile(a, (8, 1)))


def _shuffle_table(tab, ranks):
    """[ranks*128, d] -> [128, ranks*d] with [p, r*d:(r+1)*d] = row r*128+p."""
    d = tab.shape[1]
    return np.ascontiguousarray(
        tab.reshape(ranks, P, d).transpose(1, 0, 2).reshape(P, ranks * d)
    )


def kernel(node_embeddings, action_edges, ew1, eb1, ew2, eb2, aw1, ab1, aw2, ab2):
    node_embeddings = np.asarray(node_embeddings)
    action_edges = np.asarray(action_edges)
    b, n, d = node_embeddings.shape
    e = action_edges.shape[1]

    src = np.clip(action_edges[..., 0], 0, n - 1).astype(np.int64)
    tgt = np.clip(action_edges[..., 1], 0, n - 1).astype(np.int64)

    # 4 groups by (src-half, tgt-half); stable order within group
    gid = (src >= NA).astype(np.int8) * 2 + (tgt >= NA).astype(np.int8)
    orders = [np.argsort(gid[i], kind="stable") for i in range(b)]
    counts = np.stack([np.bincount(gid[i], minlength=4) for i in range(b)])
    group_sizes = [int(-(-counts[:, g].max() // SUB) * SUB) for g in range(4)]
    tp = sum(group_sizes)

    # per-core padded index streams (adjusted into region-local row ids)
    in_maps = []
    w = {
        "wse": np.ascontiguousarray(ew1[:d]).astype(BF16),
        "wte": np.ascontiguousarray(ew1[d:]).astype(BF16),
        "wsa": np.ascontiguousarray(aw1[:d]).astype(BF16),
        "wta": np.ascontiguousarray(aw1[d:]).astype(BF16),
        "w2e": np.concatenate([np.asarray(ew2), np.zeros((d, 4), np.float32)], axis=1).astype(BF16),
        "w2a": np.concatenate([np.zeros((d, 1), np.float32), np.asarray(aw2)], axis=1).astype(BF16),
        "b1e": np.asarray(eb1, np.float32).reshape(d, 1),
        "b1a": np.asarray(ab1, np.float32).reshape(d, 1),
        "b2": np.concatenate([np.asarray(eb2), np.asarray(ab2)]).astype(np.float32).reshape(5, 1),
    }
    for i in range(b):
        emb = node_embeddings[i].astype(BF16)
        tab_a = emb[:NA]
        tab_b = np.zeros((NB, d), BF16)
        tab_b[: n - NA] = emb[NA:]
        if USE_SBUF_TABLE:
            tab_a = _shuffle_table(tab_a, RANKS_A)
            tab_b = _shuffle_table(tab_b, RANKS_B)
        s_perm, t_perm = src[i][orders[i]], tgt[i][orders[i]]
        s_stream = np.zeros(tp, np.int16)
        t_stream = np.zeros(tp, np.int16)
        off = 0
        pos = 0
        for g in range(4):
            cnt = int(counts[i, g])
            s_g = s_perm[pos:pos + cnt]
            t_g = t_perm[pos:pos + cnt]
            s_stream[off:off + cnt] = (s_g - (NA if g >= 2 else 0)).astype(np.int16)
            t_stream[off:off + cnt] = (t_g - (NA if g % 2 else 0)).astype(np.int16)
            off += group_sizes[g]
            pos += cnt
        in_maps.append({
            "tabA": tab_a, "tabB": tab_b,
            "isrc": _wrap_idx(s_stream), "itgt": _wrap_idx(t_stream),
            **w,
        })

    nc = build_kernel(group_sizes)
    res = run_bass_kernel_spmd(nc, in_maps, core_ids=list(range(N_CORES)))

    edge_logits = np.empty((b, e), np.float32)
    army_logits = np.empty((b, e, 4), np.float32)
    for i in range(b):
        data = res.results[i]["out"]  # [5, tp]
        off = 0
        pos = 0
        for g in range(4):
            cnt = int(counts[i, g])
            ids = orders[i][pos:pos + cnt]
            edge_logits[i, ids] = data[0, off:off + cnt]
            army_logits[i, ids, :] = data[1:5, off:off + cnt].T
            off += group_sizes[g]
            pos += cnt

    s_raw = action_edges[..., 0]
    t_raw = action_edges[..., 1]
    invalid_self = (s_raw == t_raw) & (s_raw >= 0) & (t_raw >= 0)
    edge_logits -= invalid_self.astype(np.float32) * 100.0
    return edge_logits, army_logits
